# revision 7
# baseline (speedup 1.0000x reference)
"""BERT self-attention (B=4, S=2048, H=1024, 16 heads x 64) on 8 TRN2 NeuronCores.

Sharding: data-parallel over batch (4) x tensor-parallel over head-groups (2).
Core c handles batch c//2 and heads [8*(c%2), 8*(c%2)+8): it gets the full
hidden_states[b] plus the 512 W-columns/bias entries for its heads, and
produces out[b, :, 512*g : 512*(g+1)]. No cross-core communication.

Per-core kernel (bf16/fp16 matmuls, f32 accumulation in PSUM):
  xT   = transpose(x) via PE (bf16)               [1024h, 2048s]
  QT/KT = W.T @ xT  (+bias)                       [512hd, 2048s]
  V'   = xT.T @ Wv (+bias), 65 cols per head with an appended ones column
  per (head-pair, q-macro 512, k-chunk 128):
    scoresT[k, q] = KT_h[:, kc].T @ QT_h[:, qm]   (two heads row-packed, K=64)
    expT = exp(0.125 * scoresT)                   (ACT, fp16 out, N=1024/inst)
    ctxT[65, q] += V'_h[kc].T @ expT              (row 64 = softmax denominator)
  epilogue: ctxT -> hardware DMA-transpose -> [q, 65]; multiply by
  reciprocal denominator; batched DMA out via HWDGE.

v2 schedule (streaming start): the ACT exp stream is the global pacer
(256 x ~1.12us).  v1 had a ~90us serial phase1 (x DMA + transposes +
projections) before the first exp.  v2 interleaves phase1 INTO period 0
(hp0, qm0): x tiles stream in (DMA order: x0-3, Wq/Wk head-pair-0 column
slices, Wv, x4-15, W rest), and period 0's k-chunks are emitted per
4-tile window as their kT/V' dependencies land.  The x bf16 casts and
all W staging casts move off the ACT engine (DVE for the early ones,
GPSIMD for the rest) so ACT does nothing but exp; a dummy exp at t=0
preloads the ACT exp table (~2.7us) off the critical path.
Measured on TRN2: see HISTORY at bottom.
"""

import sys
import types

sys.path.insert(0, "/opt/trn_rl_repo")

import numpy as np

import concourse.bass as bass
import concourse.tile as tile
from concourse import bacc, mybir
from concourse.bass_utils import run_bass_kernel_spmd
from concourse.masks import make_identity

B, S, H = 4, 2048, 1024
NH, HD = 16, 64
NCORES = 8
HEADS_PER_CORE = NH // 2      # 8 heads per core
HG = HEADS_PER_CORE * HD      # 512 = per-core head width
P = 128
QM = 512                      # q macro-tile
N_QM = S // QM                # 4
N_KC = S // P                 # 16 k chunks
N_ST = S // P                 # 16 s tiles
N_HB = H // P                 # 8 h chunks (contraction)
N_MT = HG // P                # 4 hd m-tiles

FP32 = mybir.dt.float32
BF16 = mybir.dt.bfloat16
FP16 = mybir.dt.float16


def _ensure_profile_hook():
    """The image's antenv lacks axon_hooks; shim it so trace=True works."""
    try:
        from antenv.axon_hooks import get_axon_ntff_profile_hook  # noqa: F401
        return
    except ImportError:
        pass
    try:
        from trn_agent_boot.trn_boot import _ntff_profile_via_ctypes
    except ImportError:
        return
    hook = _ntff_profile_via_ctypes("/opt/axon/libaxon_pjrt.so")
    mod = types.ModuleType("antenv.axon_hooks")
    mod.get_axon_ntff_profile_hook = lambda: hook
    mod.set_axon_ntff_profile_hook = lambda h: None
    sys.modules["antenv.axon_hooks"] = mod


def build():
    nc = bacc.Bacc("TRN2", target_bir_lowering=False, debug=False,
                   num_devices=NCORES)

    x_d = nc.declare_dram_parameter("x", [S, H], FP32, isOutput=False)
    wq_d = nc.declare_dram_parameter("wq", [H, HG], FP32, isOutput=False)
    wk_d = nc.declare_dram_parameter("wk", [H, HG], FP32, isOutput=False)
    wv_d = nc.declare_dram_parameter("wv", [H, HG], FP32, isOutput=False)
    bq_d = nc.declare_dram_parameter("bq", [HG], FP32, isOutput=False)
    bk_d = nc.declare_dram_parameter("bk", [HG], FP32, isOutput=False)
    bv_d = nc.declare_dram_parameter("bv", [HG], FP32, isOutput=False)
    out_d = nc.declare_dram_parameter("out", [S, HG], FP32, isOutput=True)

    with tile.TileContext(nc) as tc:
        _build_body(nc, tc, x_d, (wq_d, wk_d, wv_d), (bq_d, bk_d, bv_d), out_d)

    nc.finalize()
    return nc


def _build_body(nc, tc, x_d, w_d, b_d, out_d):
    wq_d, wk_d, wv_d = w_d
    bq_d, bk_d, bv_d = b_d

    import contextlib
    from contextlib import nullcontext as _nullcontext
    ctx = contextlib.ExitStack()
    with ctx:
        const = ctx.enter_context(tc.tile_pool(name="const", bufs=1))
        xf = ctx.enter_context(tc.tile_pool(name="xf", bufs=6))
        xbp = ctx.enter_context(tc.tile_pool(name="xbp", bufs=3))
        big = ctx.enter_context(tc.tile_pool(name="big", bufs=1))
        wstage = ctx.enter_context(tc.tile_pool(name="wstage", bufs=4))
        expp = ctx.enter_context(tc.tile_pool(name="expp", bufs=10))
        epil = ctx.enter_context(tc.tile_pool(name="epil", bufs=4))
        outp = ctx.enter_context(tc.tile_pool(name="outp", bufs=4))
        # PSUM budget (8 banks): ps_sc = 2 x 2-bank slots (scores double
        # buffer), ps_ctx = 2 x 1-bank slots (ctx accumulators), ps_pj =
        # 2 x 1-bank slots (V'/QK projection accumulators).
        ps_sc = ctx.enter_context(
            tc.tile_pool(name="ps_sc", bufs=2, space="PSUM"))
        ps_ctx = ctx.enter_context(
            tc.tile_pool(name="ps_ctx", bufs=2, space="PSUM"))
        ps_pj = ctx.enter_context(
            tc.tile_pool(name="ps_pj", bufs=2, space="PSUM"))

        # ---- x prefetch: first window now, rest after the W DMAs below --
        x_tiles = {}

        def fetch_x(st):
            xt = xf.tile([P, H], FP32, tag="x", name=f"x{st}")
            nc.sync.dma_start(out=xt, in_=x_d.ap()[st * P:(st + 1) * P, :])
            x_tiles[st] = xt

        for st in range(4):
            fetch_x(st)

        # ---- constants ------------------------------------------------
        ident_b128 = const.tile([P, P], BF16)
        make_identity(nc, ident_b128)
        ident_h = const.tile([HD + 1, HD + 1], FP16)
        make_identity(nc, ident_h)
        ident_b = ident_h
        # dummy exp: forces the ACT exp table load (~2.7us) at t=0, off
        # the critical path of the first real exp.
        zin = const.tile([P, 8], FP32)
        nc.vector.memset(zin, 0.0)
        zout = const.tile([P, 8], FP16)
        nc.scalar.activation(out=zout, in_=zin,
                             func=mybir.ActivationFunctionType.Exp,
                             scale=0.125)

        # ---- W staging: DMA order decides arrival under contention -----
        w_sb = {}
        for name in ("q", "k", "v"):
            w_sb[name] = big.tile([P, N_HB, HG], BF16, tag=f"w{name}",
                                  name=f"w{name}")

        def dma_w_cols(name, wd, mt):
            # one head-pair column slice [1024, 128] of W, staged f32
            stg = wstage.tile([P, N_HB, P], FP32, tag="wstg",
                              name=f"w{name}c{mt}")
            nc.sync.dma_start(
                out=stg,
                in_=wd.ap()[:, mt * P:(mt + 1) * P].rearrange(
                    "(k p) c -> p k c", p=P),
            )
            return stg

        def cast_w_cols(name, mt, stg, eng):
            eng.tensor_copy(out=w_sb[name][:, :, mt * P:(mt + 1) * P],
                            in_=stg)

        stg_q0 = dma_w_cols("q", wq_d, 0)
        stg_k0 = dma_w_cols("k", wk_d, 0)

        bqT = const.tile([P, N_MT], FP32)
        nc.sync.dma_start(out=bqT, in_=bq_d.ap().rearrange("(o p) -> p o", p=P))
        bkT = const.tile([P, N_MT], FP32)
        nc.sync.dma_start(out=bkT, in_=bk_d.ap().rearrange("(o p) -> p o", p=P))
        bv_ap = bv_d.ap()
        bvb = const.tile([P, HG], FP32)
        nc.sync.dma_start(
            out=bvb,
            in_=bass.AP(tensor=bv_ap.tensor, offset=bv_ap.offset,
                        ap=[[0, P]] + [list(a) for a in bv_ap.ap]),
        )

        stg_wv = []
        for k in range(N_HB):
            stg = wstage.tile([P, HG], FP32, tag="wstg", name=f"wv{k}")
            nc.sync.dma_start(out=stg, in_=wv_d.ap()[k * P:(k + 1) * P, :])
            stg_wv.append(stg)

        # mt1 q/k slices before the x tail: the crawl's early projection
        # groups for head-pair 1 read them (~1MB extra ahead of x4-15 is
        # free; the crawl is PE-bound, not DMA-bound).
        stg_q1 = dma_w_cols("q", wq_d, 1)
        stg_k1 = dma_w_cols("k", wk_d, 1)

        for st in range(4, N_ST):
            fetch_x(st)

        # ---- big SBUF tensors ------------------------------------------
        xT = big.tile([P, N_HB, S], BF16, tag="xT")
        vp = big.tile([P, N_ST, HEADS_PER_CORE, HD + 1], BF16, tag="vp")
        # only the appended ones-column needs initializing; cols 0:HD are
        # fully written by the V' bias add below
        nc.vector.memset(vp[:, :, :, HD:HD + 1], 1.0)

        qT = big.tile([P, N_MT, S], BF16, tag="qT")
        kT = big.tile([P, N_MT, S], BF16, tag="kT")

        def transpose_st(st, cast_eng):
            xt = x_tiles[st]
            xb = xbp.tile([P, H], BF16, tag="xb", name=f"xb{st}")
            cast_eng.tensor_copy(out=xb, in_=xt)
            for half in range(2):
                ps = ps_sc.tile([P, 4, P], BF16, tag="sc", name=f"xt{st}{half}")
                for q in range(4):
                    hb = half * 4 + q
                    nc.tensor.transpose(
                        ps[:, q, :], xb[:, hb * P:(hb + 1) * P], ident_b128)
                nc.vector.tensor_copy(
                    out=xT[:, half * 4:half * 4 + 4, st * P:(st + 1) * P],
                    in_=ps,
                )

        def proj_one(w_name, mt, n, pool=None, tag=None):
            pool = pool or ps_pj
            tag = tag or "pj"
            dst, bias = (qT, bqT) if w_name == "q" else (kT, bkT)
            ps = pool.tile([P, QM], FP32, tag=tag, name=f"proj{w_name}{mt}{n}")
            for k in range(N_HB):
                nc.tensor.matmul(
                    ps,
                    lhsT=w_sb[w_name][:, k, mt * P:(mt + 1) * P],
                    rhs=xT[:, k, n * QM:(n + 1) * QM],
                    start=(k == 0),
                    stop=(k == N_HB - 1),
                )
            nc.vector.tensor_scalar_add(
                out=dst[:, mt, n * QM:(n + 1) * QM],
                in0=ps,
                scalar1=bias[:, mt:mt + 1],
            )

        def vprime(st):
            psv = ps_pj.tile([P, HG], FP32, tag="pj", name=f"v{st}")
            for hb in range(N_HB):
                nc.tensor.matmul(
                    psv,
                    lhsT=xT[:, hb, st * P:(st + 1) * P],
                    rhs=w_sb["v"][:, hb, :],
                    start=(hb == 0),
                    stop=(hb == N_HB - 1),
                )
            nc.vector.scalar_tensor_tensor(
                out=vp[:, st, :, 0:HD],
                in0=psv.rearrange("p (h d) -> p h d", h=HEADS_PER_CORE),
                scalar=1.0,
                in1=bvb.rearrange("p (h d) -> p h d", h=HEADS_PER_CORE),
                op0=mybir.AluOpType.mult,
                op1=mybir.AluOpType.add,
            )

        # ---- attention primitives --------------------------------------
        PD = 80  # 65 padded to a multiple of XBAR_TILE_SRC_ROWS (16)

        def new_ctx_ps(hp, qm):
            return [ps_ctx.tile([HD + 1, QM], FP32, tag="ctx",
                                name=f"ctx{hp}{qm}{hh}")
                    for hh in range(2)]

        def score_exp(hp, qm, kc):
            sc = ps_sc.tile([P, 2, QM], FP32, tag="sc",
                            name=f"sc{hp}{qm}{kc}")
            for hh in range(2):
                lo = hh * HD
                nc.tensor.matmul(
                    sc[:, hh, :],
                    lhsT=kT[lo:lo + HD, hp, kc * P:(kc + 1) * P],
                    rhs=qT[lo:lo + HD, hp, qm * QM:(qm + 1) * QM],
                    start=True,
                    stop=True,
                    tile_position=(lo, 0),
                )
            et = expp.tile([P, 2, QM], FP16, tag="exp")
            nc.scalar.activation(
                out=et, in_=sc,
                func=mybir.ActivationFunctionType.Exp,
                scale=0.125,
            )
            return et

        def ctx_mm(hp, kc, ctx_ps, et):
            for hh in range(2):
                nc.tensor.matmul(
                    ctx_ps[hh],
                    lhsT=vp[:, kc, 2 * hp + hh, :],
                    rhs=et[:, hh, :],
                    start=(kc == 0),
                    stop=(kc == N_KC - 1),
                )

        # The Q/K projections for later head-pairs are paced globally at
        # ~7 matmuls per 8 iterations across the steady periods: the exp
        # instruction only runs at its pure rate when per-iteration PE
        # work stays at or below it.
        def make_proj_state():
            queue = []
            for nxt in (1, 2, 3):
                queue.append(("q", nxt, 0))
                queue.extend(("k", nxt, n) for n in range(N_QM))
                queue.extend(("q", nxt, n) for n in (1, 2, 3))
            return {"queue": queue, "ci": 0, "mi": 0, "tile": None}

        def attn_kc(hp, qm, kc, ctx_ps, pstate):
            # Hoist the first few chunks' proj+scores to the period start
            # so the PE (and the ACT's score feed) stays saturated while
            # the previous period's ctx backlog drains.
            hoist = (tc.high_priority(offset=6 * kc) if kc in (1, 2, 3, 4)
                     else _nullcontext())
            with hoist:
                if kc % 8 != 7 and pstate["ci"] < len(pstate["queue"]):
                    w_name, nxt, n = pstate["queue"][pstate["ci"]]
                    mi = pstate["mi"]
                    if mi == 0:
                        pstate["tile"] = ps_pj.tile(
                            [P, QM], FP32, tag="pj",
                            name=f"pj{w_name}{nxt}{n}")
                    nc.tensor.matmul(
                        pstate["tile"],
                        lhsT=w_sb[w_name][:, mi, nxt * P:(nxt + 1) * P],
                        rhs=xT[:, mi, n * QM:(n + 1) * QM],
                        start=(mi == 0),
                        stop=(mi == N_HB - 1),
                    )
                    if mi == N_HB - 1:
                        dst, bias = (qT, bqT) if w_name == "q" else (kT, bkT)
                        nc.vector.tensor_scalar_add(
                            out=dst[:, nxt, n * QM:(n + 1) * QM],
                            in0=pstate["tile"],
                            scalar1=bias[:, nxt:nxt + 1],
                        )
                        pstate["ci"] += 1
                        pstate["mi"] = 0
                    else:
                        pstate["mi"] = mi + 1
                et = score_exp(hp, qm, kc)
            ctx_mm(hp, kc, ctx_ps, et)

        def epilogue_a(ctx_ps, use_pe, ep_idx):
            """Drain the ctx accumulators out of PSUM and kick off the
            transposes. Emitted at the period boundary so the PSUM slots
            free quickly for the next period's accumulation."""
            tfulls = []
            for hh in range(2):
                csb = epil.tile([PD, QM], FP16, tag="ctxsb")
                if ep_idx < 2:
                    # rows 65:PD feed the xbar transpose as padding; each
                    # of the 4 ring buffers only needs zeroing once.
                    nc.vector.memset(csb[64:PD, :], 0.0)
                nc.vector.tensor_copy(out=csb[0:HD + 1, :], in_=ctx_ps[hh])
                if use_pe:
                    tp = ps_pj.tile([P, QM // P, HD + 2], FP16, tag="pj",
                                    name=f"tp{hh}")
                    for qs in range(QM // P):
                        nc.tensor.transpose(
                            tp[:, qs, 0:HD + 1],
                            csb[0:HD + 1, qs * P:(qs + 1) * P],
                            ident_b,
                        )
                    tfull = epil.tile([P, QM // P, HD + 1], FP16, tag="tpe")
                    nc.vector.tensor_copy(out=tfull, in_=tp[:, :, 0:HD + 1])
                else:
                    tfull = epil.tile([P, QM // P, PD], FP16, tag="tpsb")
                    for qs in range(QM // P):
                        nc.sync.dma_start_transpose(
                            out=tfull[:, qs, :],
                            in_=csb[:, qs * P:(qs + 1) * P],
                        )
                tfulls.append(tfull)
            return tfulls

        def epilogue_b(hp, qm, tfulls):
            """Reciprocal + scale + store. Deferred one period so these DVE
            ops are emitted after the next period's proj bias-adds and can
            never head-of-line-block them."""
            for hh in range(2):
                tfull = tfulls[hh]
                rc = outp.tile([P, QM // P], FP32, tag="recip")
                nc.vector.reciprocal(out=rc, in_=tfull[:, :, HD:HD + 1])
                ot = outp.tile([P, QM // P, HD], FP32, tag="out")
                for qs in range(QM // P):
                    nc.vector.tensor_scalar_mul(
                        ot[:, qs, :], tfull[:, qs, 0:HD], rc[:, qs:qs + 1])
                row = qm * QM
                col = (2 * hp + hh) * HD
                nc.sync.dma_start(
                    out=out_d.ap()[row:row + QM, col:col + HD].rearrange(
                        "(a p) c -> p a c", p=P),
                    in_=ot,
                )

        # ---- period 0 (hp0, qm0): streaming crawl ----------------------
        # Window 0: transposes for x0-3, Q/K projections for head-pair 0,
        # scores+exps for kc0-3.  ctx is deferred one window (the V' weight
        # DMA lands after the W q/k slices; emitting ctx earlier would
        # head-of-line-block the PE stream on it).
        for st in range(4):
            transpose_st(st, nc.vector)
        cast_w_cols("q", 0, stg_q0, nc.vector)
        cast_w_cols("k", 0, stg_k0, nc.vector)
        proj_one("q", 0, 0)
        proj_one("k", 0, 0)
        ctx_ps0 = new_ctx_ps(0, 0)
        ets0 = [score_exp(0, 0, kc) for kc in range(4)]
        for k, stg in enumerate(stg_wv):
            nc.gpsimd.tensor_copy(out=w_sb["v"][:, k, :], in_=stg)
        for st in range(4):
            vprime(st)
        for kc in range(4):
            ctx_mm(0, kc, ctx_ps0, ets0[kc])
        del ets0
        # Windows 1-3: per 4-tile window, transposes + V' as x arrives,
        # then this window's kT (and the qT of a later period), then
        # score/exp/ctx for its 4 k-chunks.  Each window also emits one
        # full later-head-pair projection group: the steady-state pace
        # (7 matmuls per 8 iters over 240 iters) would otherwise emit the
        # kT mt3 writes AFTER the period-12 scores that read them.
        pstate = make_proj_state()
        cast_w_cols("q", 1, stg_q1, nc.gpsimd)
        cast_w_cols("k", 1, stg_k1, nc.gpsimd)
        for w in range(1, 4):
            for st in range(4 * w, 4 * w + 4):
                transpose_st(st, nc.gpsimd)
                vprime(st)
            proj_one("k", 0, w)
            proj_one("q", 0, w)
            wn, nxt, n = pstate["queue"][pstate["ci"]]
            proj_one(wn, nxt, n)
            pstate["ci"] += 1
            for kc in range(4 * w, 4 * w + 4):
                et = score_exp(0, 0, kc)
                ctx_mm(0, kc, ctx_ps0, et)
        # stage the remaining W column slices (DMA after x15; casts on
        # GPSIMD after the xb casts)
        stg_rest = []
        for mt in (2, 3):
            stg_rest.append(("q", mt, dma_w_cols("q", wq_d, mt)))
            stg_rest.append(("k", mt, dma_w_cols("k", wk_d, mt)))
        for name, mt, stg in stg_rest:
            cast_w_cols(name, mt, stg, nc.gpsimd)

        tfulls0 = epilogue_a(ctx_ps0, use_pe=False, ep_idx=0)
        pending = (0, 0, tfulls0)

        # ---- steady periods --------------------------------------------
        for hp in range(N_MT):
            for qm in range(N_QM):
                if hp == 0 and qm == 0:
                    continue
                ctx_ps = new_ctx_ps(hp, qm)
                for kc in range(N_KC):
                    attn_kc(hp, qm, kc, ctx_ps, pstate)
                ep_idx = hp * N_QM + qm
                tfulls = epilogue_a(
                    ctx_ps,
                    use_pe=(hp == N_MT - 1 and qm == N_QM - 1),
                    ep_idx=ep_idx,
                )
                if pending is not None:
                    epilogue_b(*pending)
                pending = (hp, qm, tfulls)
        epilogue_b(*pending)


_NC_CACHE = None


def _get_nc():
    global _NC_CACHE
    if _NC_CACHE is None:
        _NC_CACHE = build()
    return _NC_CACHE


def make_in_maps(hidden_states, Wq, bq, Wk, bk, Wv, bv):
    hs = np.ascontiguousarray(np.asarray(hidden_states, dtype=np.float32))
    ws = {k: np.asarray(v, dtype=np.float32)
          for k, v in (("q", Wq), ("k", Wk), ("v", Wv))}
    bs = {k: np.asarray(v, dtype=np.float32)
          for k, v in (("q", bq), ("k", bk), ("v", bv))}
    in_maps = []
    for c in range(NCORES):
        b, g = c // 2, c % 2
        sl = slice(g * HG, (g + 1) * HG)
        in_maps.append({
            "x": np.ascontiguousarray(hs[b]),
            "wq": np.ascontiguousarray(ws["q"][:, sl]),
            "wk": np.ascontiguousarray(ws["k"][:, sl]),
            "wv": np.ascontiguousarray(ws["v"][:, sl]),
            "bq": np.ascontiguousarray(bs["q"][sl]),
            "bk": np.ascontiguousarray(bs["k"][sl]),
            "bv": np.ascontiguousarray(bs["v"][sl]),
        })
    return in_maps


def run(in_maps, trace=False):
    _ensure_profile_hook()
    nc = _get_nc()
    return run_bass_kernel_spmd(nc, in_maps, list(range(NCORES)), trace=trace)


def kernel(hidden_states, Wq, bq, Wk, bk, Wv, bv):
    in_maps = make_in_maps(hidden_states, Wq, bq, Wk, bk, Wv, bv)
    res = run(in_maps, trace=False)
    out = np.empty((B, S, H), dtype=np.float32)
    for c in range(NCORES):
        b, g = c // 2, c % 2
        out[b, :, g * HG:(g + 1) * HG] = res.results[c]["out"]
    return out


# HISTORY (HW exec time, rel l2 err):
#   v1 (serial phase1):          368139 ns / 367361 ns, 3.7e-3
#   v2 (streaming period 0):     ?


# revision 17
# speedup vs baseline: 1.0604x; 1.0604x over previous
"""BERT self-attention (B=4, S=2048, H=1024, 16 heads x 64) on 8 TRN2 NeuronCores.

Sharding: data-parallel over batch (4) x tensor-parallel over head-groups (2).
Core c handles batch c//2 and heads [8*(c%2), 8*(c%2)+8): it gets the full
hidden_states[b] plus the 512 W-columns/bias entries for its heads, and
produces out[b, :, 512*g : 512*(g+1)]. No cross-core communication.

Per-core kernel (bf16/fp16 matmuls, f32 accumulation in PSUM):
  xT   = transpose(x) via PE (bf16)               [1024h, 2048s]
  QT/KT = W.T @ xT  (+bias)                       [512hd, 2048s]
  V'   = xT.T @ Wv (+bias), 65 cols per head with an appended ones column
  per (head-pair, q-macro 512, k-chunk 128):
    scoresT[k, q] = KT_h[:, kc].T @ QT_h[:, qm]   (two heads row-packed, K=64)
    expT = exp(0.125 * scoresT)                   (ACT, fp16 out, N=1024/inst)
    ctxT[65, q] += V'_h[kc].T @ expT              (row 64 = softmax denominator)
  epilogue: ctxT -> hardware DMA-transpose -> [q, 65]; multiply by
  reciprocal denominator; batched DMA out via HWDGE.

v2 schedule (streaming start): the ACT exp stream is the global pacer
(256 x ~1.12us).  v1 had a ~90us serial phase1 (x DMA + transposes +
projections) before the first exp.  v2 interleaves phase1 INTO period 0
(hp0, qm0): x tiles stream in (DMA order: x0-3, Wq/Wk head-pair-0 column
slices, Wv, x4-15, W rest), and period 0's k-chunks are emitted per
4-tile window as their kT/V' dependencies land.  The x bf16 casts and
all W staging casts move off the ACT engine (DVE for the early ones,
GPSIMD for the rest) so ACT does nothing but exp; a dummy exp at t=0
preloads the ACT exp table (~2.7us) off the critical path.
Measured on TRN2: see HISTORY at bottom.
"""

import sys
import types

sys.path.insert(0, "/opt/trn_rl_repo")

import numpy as np

import concourse.bass as bass
import concourse.tile as tile
from concourse import bacc, mybir
from concourse.bass_utils import run_bass_kernel_spmd
from concourse.masks import make_identity

B, S, H = 4, 2048, 1024
NH, HD = 16, 64
NCORES = 8
HEADS_PER_CORE = NH // 2      # 8 heads per core
HG = HEADS_PER_CORE * HD      # 512 = per-core head width
P = 128
QM = 512                      # q macro-tile
N_QM = S // QM                # 4
N_KC = S // P                 # 16 k chunks
N_ST = S // P                 # 16 s tiles
N_HB = H // P                 # 8 h chunks (contraction)
N_MT = HG // P                # 4 hd m-tiles

FP32 = mybir.dt.float32
BF16 = mybir.dt.bfloat16
FP16 = mybir.dt.float16


def _ensure_profile_hook():
    """The image's antenv lacks axon_hooks; shim it so trace=True works."""
    try:
        from antenv.axon_hooks import get_axon_ntff_profile_hook  # noqa: F401
        return
    except ImportError:
        pass
    try:
        from trn_agent_boot.trn_boot import _ntff_profile_via_ctypes
    except ImportError:
        return
    hook = _ntff_profile_via_ctypes("/opt/axon/libaxon_pjrt.so")
    mod = types.ModuleType("antenv.axon_hooks")
    mod.get_axon_ntff_profile_hook = lambda: hook
    mod.set_axon_ntff_profile_hook = lambda h: None
    sys.modules["antenv.axon_hooks"] = mod


def build():
    nc = bacc.Bacc("TRN2", target_bir_lowering=False, debug=False,
                   num_devices=NCORES)

    x_d = nc.declare_dram_parameter("x", [S, H], FP32, isOutput=False)
    wq_d = nc.declare_dram_parameter("wq", [H, HG], FP32, isOutput=False)
    wk_d = nc.declare_dram_parameter("wk", [H, HG], FP32, isOutput=False)
    wv_d = nc.declare_dram_parameter("wv", [H, HG], FP32, isOutput=False)
    bq_d = nc.declare_dram_parameter("bq", [HG], FP32, isOutput=False)
    bk_d = nc.declare_dram_parameter("bk", [HG], FP32, isOutput=False)
    bv_d = nc.declare_dram_parameter("bv", [HG], FP32, isOutput=False)
    out_d = nc.declare_dram_parameter("out", [S, HG], FP32, isOutput=True)

    with tile.TileContext(nc) as tc:
        _build_body(nc, tc, x_d, (wq_d, wk_d, wv_d), (bq_d, bk_d, bv_d), out_d)

    nc.finalize()
    return nc


def _build_body(nc, tc, x_d, w_d, b_d, out_d):
    wq_d, wk_d, wv_d = w_d
    bq_d, bk_d, bv_d = b_d

    import contextlib
    from contextlib import nullcontext as _nullcontext
    ctx = contextlib.ExitStack()
    with ctx:
        const = ctx.enter_context(tc.tile_pool(name="const", bufs=1))
        xf = ctx.enter_context(tc.tile_pool(name="xf", bufs=2))
        xbp = ctx.enter_context(tc.tile_pool(name="xbp", bufs=2))
        big = ctx.enter_context(tc.tile_pool(name="big", bufs=1))
        wstage = ctx.enter_context(tc.tile_pool(name="wstage", bufs=4))
        expp = ctx.enter_context(tc.tile_pool(name="expp", bufs=9))
        epil = ctx.enter_context(tc.tile_pool(name="epil", bufs=4))
        outp = ctx.enter_context(tc.tile_pool(name="outp", bufs=3))
        # PSUM budget (8 banks): ps_sc = 2 x 2-bank slots (scores double
        # buffer), ps_ctx = 2 x 1-bank slots (ctx accumulators), ps_pj =
        # 2 x 1-bank slots (V'/QK projection accumulators).
        ps_sc = ctx.enter_context(
            tc.tile_pool(name="ps_sc", bufs=2, space="PSUM"))
        ps_ctx = ctx.enter_context(
            tc.tile_pool(name="ps_ctx", bufs=2, space="PSUM"))
        ps_pj = ctx.enter_context(
            tc.tile_pool(name="ps_pj", bufs=2, space="PSUM"))

        # ---- x prefetch: 2MB window blocks (few big DMAs — each enqueue
        # costs ~0.6-1us on the Sync engine) -----------------------------
        x_blks = {}

        def fetch_xblk(w):
            xt = xf.tile([P, 4, H], FP32, tag="x", name=f"xblk{w}")
            nc.sync.dma_start(
                out=xt,
                in_=x_d.ap()[4 * w * P:4 * (w + 1) * P, :].rearrange(
                    "(a p) c -> p a c", p=P),
            )
            x_blks[w] = xt

        fetch_xblk(0)

        # ---- constants ------------------------------------------------
        ident_b128 = const.tile([P, P], BF16)
        make_identity(nc, ident_b128)
        ident_h = const.tile([HD + 1, HD + 1], FP16)
        make_identity(nc, ident_h)
        ident_b = ident_h
        # dummy exp: forces the ACT exp table load (~2.7us) at t=0, off
        # the critical path of the first real exp.
        zin = const.tile([P, 8], FP32)
        nc.vector.memset(zin, 0.0)
        zout = const.tile([P, 8], FP16)
        nc.scalar.activation(out=zout, in_=zin,
                             func=mybir.ActivationFunctionType.Exp,
                             scale=0.125)

        # ---- W staging: DMA order decides arrival under contention -----
        w_sb = {}
        for name in ("q", "k", "v"):
            w_sb[name] = big.tile([P, N_HB, HG], BF16, tag=f"w{name}",
                                  name=f"w{name}")

        def dma_w_cols(name, wd, mt):
            # one head-pair column slice [1024, 128] of W, staged f32
            stg = wstage.tile([P, N_HB, P], FP32, tag="wstg",
                              name=f"w{name}c{mt}")
            nc.sync.dma_start(
                out=stg,
                in_=wd.ap()[:, mt * P:(mt + 1) * P].rearrange(
                    "(k p) c -> p k c", p=P),
            )
            return stg

        def cast_w_cols(name, mt, stg, eng):
            eng.tensor_copy(out=w_sb[name][:, :, mt * P:(mt + 1) * P],
                            in_=stg)

        stg_q0 = dma_w_cols("q", wq_d, 0)
        stg_k0 = dma_w_cols("k", wk_d, 0)

        bqT = const.tile([P, N_MT], FP32)
        nc.sync.dma_start(out=bqT, in_=bq_d.ap().rearrange("(o p) -> p o", p=P))
        bkT = const.tile([P, N_MT], FP32)
        nc.sync.dma_start(out=bkT, in_=bk_d.ap().rearrange("(o p) -> p o", p=P))
        # bv broadcast built on-chip (a [0,P]-stride broadcast DMA costs
        # ~4us of HWDGE descriptor generation): K=1 fp32 matmul of
        # ones[1,P].T @ bv[1,HG] into PSUM, then one DVE copy.
        bv_ap = bv_d.ap()
        bv_row = const.tile([1, HG], FP32)
        nc.sync.dma_start(
            out=bv_row,
            in_=bass.AP(tensor=bv_ap.tensor, offset=bv_ap.offset,
                        ap=[[0, 1]] + [list(a) for a in bv_ap.ap]),
        )
        ones1 = const.tile([1, P], FP32)
        nc.vector.memset(ones1, 1.0)
        bvb = const.tile([P, HG], FP32)
        ps_bv = ps_pj.tile([P, HG], FP32, tag="pj", name="ps_bv")
        nc.tensor.matmul(ps_bv, lhsT=ones1, rhs=bv_row, start=True, stop=True)
        nc.vector.tensor_copy(out=bvb, in_=ps_bv)

        # wv as one 2MB DMA
        stg_wv = wstage.tile([P, N_HB, HG], FP32, tag="wstgv", bufs=1,
                             name="wvstg")
        nc.sync.dma_start(
            out=stg_wv,
            in_=wv_d.ap().rearrange("(k p) c -> p k c", p=P),
        )

        # mt1 q/k slices before the x tail: the crawl's early projection
        # groups for head-pair 1 read them (~1MB extra ahead of the x tail
        # is free; the crawl is PE-bound, not DMA-bound).
        stg_q1 = dma_w_cols("q", wq_d, 1)
        stg_k1 = dma_w_cols("k", wk_d, 1)

        for w in range(1, 4):
            fetch_xblk(w)

        # ---- big SBUF tensors ------------------------------------------
        xT = big.tile([P, N_HB, S], BF16, tag="xT")
        vp = big.tile([P, N_ST, HEADS_PER_CORE, HD + 1], BF16, tag="vp")
        # only the appended ones-column needs initializing; cols 0:HD are
        # fully written by the V' bias add below
        nc.vector.memset(vp[:, :, :, HD:HD + 1], 1.0)

        qT = big.tile([P, N_MT, S], BF16, tag="qT")
        kT = big.tile([P, N_MT, S], BF16, tag="kT")

        def transpose_st(st):
            # cast on DVE; the PSUM->SBUF transpose drains ride the ACT
            # engine (idle during the crawl; GPSIMD copies are ~5x slower
            # than DVE and DVE is near-saturated in the crawl).
            xt = x_blks[st // 4][:, st % 4, :]
            xb = xbp.tile([P, H], BF16, tag="xb", name=f"xb{st}")
            nc.vector.tensor_copy(out=xb, in_=xt)
            for half in range(2):
                ps = ps_sc.tile([P, 4, P], BF16, tag="sc", name=f"xt{st}{half}")
                for q in range(4):
                    hb = half * 4 + q
                    nc.tensor.transpose(
                        ps[:, q, :], xb[:, hb * P:(hb + 1) * P], ident_b128)
                nc.scalar.copy(
                    out=xT[:, half * 4:half * 4 + 4, st * P:(st + 1) * P],
                    in_=ps,
                )

        def proj_one(w_name, mt, n, pool=None, tag=None):
            pool = pool or ps_pj
            tag = tag or "pj"
            dst, bias = (qT, bqT) if w_name == "q" else (kT, bkT)
            ps = pool.tile([P, QM], FP32, tag=tag, name=f"proj{w_name}{mt}{n}")
            for k in range(N_HB):
                nc.tensor.matmul(
                    ps,
                    lhsT=w_sb[w_name][:, k, mt * P:(mt + 1) * P],
                    rhs=xT[:, k, n * QM:(n + 1) * QM],
                    start=(k == 0),
                    stop=(k == N_HB - 1),
                )
            nc.vector.tensor_scalar_add(
                out=dst[:, mt, n * QM:(n + 1) * QM],
                in0=ps,
                scalar1=bias[:, mt:mt + 1],
            )

        def vprime(st):
            psv = ps_pj.tile([P, HG], FP32, tag="pj", name=f"v{st}")
            for hb in range(N_HB):
                nc.tensor.matmul(
                    psv,
                    lhsT=xT[:, hb, st * P:(st + 1) * P],
                    rhs=w_sb["v"][:, hb, :],
                    start=(hb == 0),
                    stop=(hb == N_HB - 1),
                )
            nc.vector.scalar_tensor_tensor(
                out=vp[:, st, :, 0:HD],
                in0=psv.rearrange("p (h d) -> p h d", h=HEADS_PER_CORE),
                scalar=1.0,
                in1=bvb.rearrange("p (h d) -> p h d", h=HEADS_PER_CORE),
                op0=mybir.AluOpType.mult,
                op1=mybir.AluOpType.add,
            )

        # ---- attention primitives --------------------------------------
        PD = 80  # 65 padded to a multiple of XBAR_TILE_SRC_ROWS (16)

        def new_ctx_ps(hp, qm):
            return [ps_ctx.tile([HD + 1, QM], FP32, tag="ctx",
                                name=f"ctx{hp}{qm}{hh}")
                    for hh in range(2)]

        def score_exp(hp, qm, kc):
            sc = ps_sc.tile([P, 2, QM], FP32, tag="sc",
                            name=f"sc{hp}{qm}{kc}")
            for hh in range(2):
                lo = hh * HD
                nc.tensor.matmul(
                    sc[:, hh, :],
                    lhsT=kT[lo:lo + HD, hp, kc * P:(kc + 1) * P],
                    rhs=qT[lo:lo + HD, hp, qm * QM:(qm + 1) * QM],
                    start=True,
                    stop=True,
                    tile_position=(lo, 0),
                )
            et = expp.tile([P, 2, QM], FP16, tag="exp")
            nc.scalar.activation(
                out=et, in_=sc,
                func=mybir.ActivationFunctionType.Exp,
                scale=0.125,
            )
            return et

        def ctx_mm(hp, kc, ctx_ps, et):
            for hh in range(2):
                nc.tensor.matmul(
                    ctx_ps[hh],
                    lhsT=vp[:, kc, 2 * hp + hh, :],
                    rhs=et[:, hh, :],
                    start=(kc == 0),
                    stop=(kc == N_KC - 1),
                )

        # The Q/K projections for later head-pairs are paced globally at
        # ~7 matmuls per 8 iterations across the steady periods: the exp
        # instruction only runs at its pure rate when per-iteration PE
        # work stays at or below it.
        def make_proj_state():
            queue = []
            for nxt in (1, 2, 3):
                queue.append(("q", nxt, 0))
                queue.extend(("k", nxt, n) for n in range(N_QM))
                queue.extend(("q", nxt, n) for n in (1, 2, 3))
            return {"queue": queue, "ci": 0, "mi": 0, "tile": None}

        def attn_kc(hp, qm, kc, ctx_ps, pstate):
            # Hoist the first few chunks' proj+scores to the period start
            # so the PE (and the ACT's score feed) stays saturated while
            # the previous period's ctx backlog drains.
            hoist = (tc.high_priority(offset=6 * kc) if kc in (1, 2, 3, 4)
                     else _nullcontext())
            with hoist:
                if kc % 8 != 7 and pstate["ci"] < len(pstate["queue"]):
                    w_name, nxt, n = pstate["queue"][pstate["ci"]]
                    mi = pstate["mi"]
                    if mi == 0:
                        pstate["tile"] = ps_pj.tile(
                            [P, QM], FP32, tag="pj",
                            name=f"pj{w_name}{nxt}{n}")
                    nc.tensor.matmul(
                        pstate["tile"],
                        lhsT=w_sb[w_name][:, mi, nxt * P:(nxt + 1) * P],
                        rhs=xT[:, mi, n * QM:(n + 1) * QM],
                        start=(mi == 0),
                        stop=(mi == N_HB - 1),
                    )
                    if mi == N_HB - 1:
                        dst, bias = (qT, bqT) if w_name == "q" else (kT, bkT)
                        nc.vector.tensor_scalar_add(
                            out=dst[:, nxt, n * QM:(n + 1) * QM],
                            in0=pstate["tile"],
                            scalar1=bias[:, nxt:nxt + 1],
                        )
                        pstate["ci"] += 1
                        pstate["mi"] = 0
                    else:
                        pstate["mi"] = mi + 1
                et = score_exp(hp, qm, kc)
            ctx_mm(hp, kc, ctx_ps, et)

        def epilogue_a(ctx_ps, use_pe, ep_idx):
            """Drain the ctx accumulators out of PSUM and kick off the
            transposes. Emitted at the period boundary so the PSUM slots
            free quickly for the next period's accumulation."""
            tfulls = []
            for hh in range(2):
                csb = epil.tile([PD, QM], FP16, tag="ctxsb")
                if ep_idx < 2:
                    # rows 65:PD feed the xbar transpose as padding; each
                    # of the 4 ring buffers only needs zeroing once.
                    nc.vector.memset(csb[64:PD, :], 0.0)
                nc.vector.tensor_copy(out=csb[0:HD + 1, :], in_=ctx_ps[hh])
                if use_pe:
                    tp = ps_pj.tile([P, QM // P, HD + 2], FP16, tag="pj",
                                    name=f"tp{hh}")
                    for qs in range(QM // P):
                        nc.tensor.transpose(
                            tp[:, qs, 0:HD + 1],
                            csb[0:HD + 1, qs * P:(qs + 1) * P],
                            ident_b,
                        )
                    tfull = epil.tile([P, QM // P, HD + 1], FP16, tag="tpe")
                    nc.vector.tensor_copy(out=tfull, in_=tp[:, :, 0:HD + 1])
                else:
                    tfull = epil.tile([P, QM // P, PD], FP16, tag="tpsb")
                    for qs in range(QM // P):
                        nc.sync.dma_start_transpose(
                            out=tfull[:, qs, :],
                            in_=csb[:, qs * P:(qs + 1) * P],
                        )
                tfulls.append(tfull)
            return tfulls

        def epilogue_b(hp, qm, tfulls):
            """Reciprocal + scale + store. Deferred one period so these DVE
            ops are emitted after the next period's proj bias-adds and can
            never head-of-line-block them."""
            for hh in range(2):
                tfull = tfulls[hh]
                rc = outp.tile([P, QM // P], FP32, tag="recip")
                nc.vector.reciprocal(out=rc, in_=tfull[:, :, HD:HD + 1])
                ot = outp.tile([P, QM // P, HD], FP32, tag="out")
                for qs in range(QM // P):
                    nc.vector.tensor_scalar_mul(
                        ot[:, qs, :], tfull[:, qs, 0:HD], rc[:, qs:qs + 1])
                row = qm * QM
                col = (2 * hp + hh) * HD
                nc.sync.dma_start(
                    out=out_d.ap()[row:row + QM, col:col + HD].rearrange(
                        "(a p) c -> p a c", p=P),
                    in_=ot,
                )

        # ---- period 0 (hp0, qm0): streaming crawl ----------------------
        # Window 0: transposes for x0-3, Q/K projections for head-pair 0,
        # scores+exps for kc0-3.  ctx is deferred one window (the V' weight
        # DMA lands after the W q/k slices; emitting ctx earlier would
        # head-of-line-block the PE stream on it).
        for st in range(4):
            transpose_st(st)
        cast_w_cols("q", 0, stg_q0, nc.vector)
        cast_w_cols("k", 0, stg_k0, nc.vector)
        proj_one("q", 0, 0)
        proj_one("k", 0, 0)
        ctx_ps0 = new_ctx_ps(0, 0)
        ets0 = [score_exp(0, 0, kc) for kc in range(4)]
        for k2 in range(0, N_HB, 2):
            nc.vector.tensor_copy(out=w_sb["v"][:, k2:k2 + 2, :],
                                  in_=stg_wv[:, k2:k2 + 2, :])
        for st in range(4):
            vprime(st)
        for kc in range(4):
            ctx_mm(0, kc, ctx_ps0, ets0[kc])
        del ets0
        # Windows 1-3: per 4-tile window, transposes + V' as x arrives,
        # then this window's kT (and the qT of a later period), then
        # score/exp/ctx for its 4 k-chunks.  Each window also emits one
        # full later-head-pair projection group: the steady-state pace
        # (7 matmuls per 8 iters over 240 iters) would otherwise emit the
        # kT mt3 writes AFTER the period-12 scores that read them.
        pstate = make_proj_state()
        cast_w_cols("q", 1, stg_q1, nc.vector)
        cast_w_cols("k", 1, stg_k1, nc.vector)
        for w in range(1, 4):
            for st in range(4 * w, 4 * w + 4):
                transpose_st(st)
                vprime(st)
            proj_one("k", 0, w)
            proj_one("q", 0, w)
            wn, nxt, n = pstate["queue"][pstate["ci"]]
            proj_one(wn, nxt, n)
            pstate["ci"] += 1
            for kc in range(4 * w, 4 * w + 4):
                et = score_exp(0, 0, kc)
                ctx_mm(0, kc, ctx_ps0, et)
        # stage the remaining W column slices (DMA after x15; casts on
        # GPSIMD after the xb casts)
        stg_rest = []
        for mt in (2, 3):
            stg_rest.append(("q", mt, dma_w_cols("q", wq_d, mt)))
            stg_rest.append(("k", mt, dma_w_cols("k", wk_d, mt)))
        for name, mt, stg in stg_rest:
            cast_w_cols(name, mt, stg, nc.vector)

        tfulls0 = epilogue_a(ctx_ps0, use_pe=False, ep_idx=0)
        pending = (0, 0, tfulls0)

        # ---- steady periods --------------------------------------------
        for hp in range(N_MT):
            for qm in range(N_QM):
                if hp == 0 and qm == 0:
                    continue
                ctx_ps = new_ctx_ps(hp, qm)
                for kc in range(N_KC):
                    attn_kc(hp, qm, kc, ctx_ps, pstate)
                ep_idx = hp * N_QM + qm
                tfulls = epilogue_a(
                    ctx_ps,
                    use_pe=(hp == N_MT - 1 and qm == N_QM - 1),
                    ep_idx=ep_idx,
                )
                if pending is not None:
                    epilogue_b(*pending)
                pending = (hp, qm, tfulls)
        epilogue_b(*pending)


_NC_CACHE = None


def _get_nc():
    global _NC_CACHE
    if _NC_CACHE is None:
        _NC_CACHE = build()
    return _NC_CACHE


def make_in_maps(hidden_states, Wq, bq, Wk, bk, Wv, bv):
    hs = np.ascontiguousarray(np.asarray(hidden_states, dtype=np.float32))
    ws = {k: np.asarray(v, dtype=np.float32)
          for k, v in (("q", Wq), ("k", Wk), ("v", Wv))}
    bs = {k: np.asarray(v, dtype=np.float32)
          for k, v in (("q", bq), ("k", bk), ("v", bv))}
    in_maps = []
    for c in range(NCORES):
        b, g = c // 2, c % 2
        sl = slice(g * HG, (g + 1) * HG)
        in_maps.append({
            "x": np.ascontiguousarray(hs[b]),
            "wq": np.ascontiguousarray(ws["q"][:, sl]),
            "wk": np.ascontiguousarray(ws["k"][:, sl]),
            "wv": np.ascontiguousarray(ws["v"][:, sl]),
            "bq": np.ascontiguousarray(bs["q"][sl]),
            "bk": np.ascontiguousarray(bs["k"][sl]),
            "bv": np.ascontiguousarray(bs["v"][sl]),
        })
    return in_maps


def run(in_maps, trace=False):
    _ensure_profile_hook()
    nc = _get_nc()
    return run_bass_kernel_spmd(nc, in_maps, list(range(NCORES)), trace=trace)


def kernel(hidden_states, Wq, bq, Wk, bk, Wv, bv):
    in_maps = make_in_maps(hidden_states, Wq, bq, Wk, bk, Wv, bv)
    res = run(in_maps, trace=False)
    out = np.empty((B, S, H), dtype=np.float32)
    for c in range(NCORES):
        b, g = c // 2, c % 2
        out[b, :, g * HG:(g + 1) * HG] = res.results[c]["out"]
    return out


# HISTORY (HW exec time, rel l2 err):
#   v1 (serial phase1):          368139 ns / 367361 ns, 3.7e-3
#   v2 (streaming period 0):     ?
